# revision 2
# baseline (speedup 1.0000x reference)
"""GATv2 GNN message-passing kernel for 8 Trainium2 NeuronCores.

Edge-parallel over dst-sorted edges, node-range sharded (2500 nodes/core,
groups of <=127 dst nodes / <=2048 edge slots). Device reads one packed
bf16 blob per group (eaT | xsT | ohT one-hot | xoT | dsT) which is staged
from the ExternalInput DRAM region into Internal DRAM with two large
DRAM->DRAM copies first (per-tile reads from ExternalInput DRAM measure
~5x slower when concurrent with compute on this runtime; Internal-DRAM
reads run at full rate). All group DMAs and all compute are fully
statically unrolled -- dynamic-offset DMAs inside a hardware loop cost
~260us each here, which was the previous version's entire runtime.

Per group the device computes, wide-phase to keep cross-engine
dependency chains at stage granularity instead of per-chunk:
  per 128-edge chunk: PE matmuls e=eaT@We + xr-gather(ohT@xr) -> psa,
  xl=xsT@Wl -> psb; ACT copies psb->xls_all; DVE adds z0_all=psa+xls.
  then ONE wide op per stage: prelu (ACT), att-mult (Pool), head-reduce
  (DVE), exp (ACT), es*xl (DVE); finally 16 back-to-back one-hot
  aggregation matmuls into a [span, 320 num | 10 den] PSUM accumulator.
Self-loops (fill_value='mean'), softmax divide, head-mean, relu, pool
and the [32,2] head run on host (O(N*HC) numpy vs O(E*HC) on device);
the (bl+br) bias rides in xr, bl commutes through the softmax and is
added after the divide.
"""

import sys

sys.path.insert(0, "/opt/trn_rl_repo")

import numpy as np
import ml_dtypes

N = 20000
E = 320000
FIN = 128
EDIM = 128
H = 10
C = 32
B = 64
EPS = 1e-5
NEG_SLOPE = 0.2
NCORES = 8
NODES_PER_CORE = N // NCORES
GE = 2048          # edge slots per group
SPAN = 127         # max nodes per group (row 127 of psc is trash)
CHUNKS = GE // 128
HC = H * C          # 320
MPW = HC + H        # 330: msg | escore columns
P = 128
# blob layout (bf16 cols): eaT | xsT | ohT | xoT | dsT
OFF_EAT = 0
OFF_XST = GE
OFF_OHT = 2 * GE
OFF_XOT = 3 * GE
OFF_DST = 3 * GE + P
WBLOB = 3 * GE + P + CHUNKS
BF16 = ml_dtypes.bfloat16

_compiled = {}


def _build_nc(NG, reps=1, dup=1):
    import concourse.mybir as mybir
    import concourse.tile as tile
    from concourse import bacc

    dt = mybir.dt
    f32 = dt.float32
    bf16 = dt.bfloat16
    Alu = mybir.AluOpType
    Act = mybir.ActivationFunctionType

    nc = bacc.Bacc("TRN2")

    blob_t = nc.dram_tensor("blob", [NG, P, WBLOB], bf16, kind="ExternalInput")
    stage_t = nc.dram_tensor("stage", [NG, P, WBLOB], bf16, kind="Internal")
    wl_t = nc.dram_tensor("wl", [FIN, HC], bf16, kind="ExternalInput")
    wr_t = nc.dram_tensor("wr", [FIN, HC], bf16, kind="ExternalInput")
    we_t = nc.dram_tensor("we", [EDIM, HC], bf16, kind="ExternalInput")
    att_t = nc.dram_tensor("attrep", [P, HC], bf16, kind="ExternalInput")
    attw_t = nc.dram_tensor("attw", [P, CHUNKS, HC], bf16, kind="ExternalInput")
    brep_t = nc.dram_tensor("brep", [P, HC], f32, kind="ExternalInput")
    ir_t = nc.dram_tensor("iotar", [P, P], bf16, kind="ExternalInput")
    out_t = nc.dram_tensor("gpart", [NG, P, MPW], f32, kind="ExternalOutput")

    with tile.TileContext(nc) as tc:
        with (
            tc.tile_pool(name="const", bufs=1) as cp,
            tc.tile_pool(name="sb", bufs=3) as sb,
            tc.tile_pool(name="wide", bufs=2) as wd,
            tc.tile_pool(name="psA", bufs=3, space="PSUM") as pA,
            tc.tile_pool(name="psB", bufs=3, space="PSUM") as pB,
            tc.tile_pool(name="psC", bufs=2, space="PSUM") as pC,
        ):
            wl = cp.tile([FIN, HC], bf16, tag="wl", name="wl")
            nc.sync.dma_start(wl[:], wl_t[:])
            wr = cp.tile([FIN, HC], bf16, tag="wr", name="wr")
            nc.sync.dma_start(wr[:], wr_t[:])
            we = cp.tile([EDIM, HC], bf16, tag="we", name="we")
            nc.sync.dma_start(we[:], we_t[:])
            attrep = cp.tile([P, HC], bf16, tag="attrep", name="attrep")
            nc.sync.dma_start(attrep[:], att_t[:])
            attw = cp.tile([P, CHUNKS, HC], bf16, tag="attw", name="attw")
            nc.sync.dma_start(attw[:], attw_t[:])
            brep = cp.tile([P, HC], f32, tag="brep", name="brep")
            nc.sync.dma_start(brep[:], brep_t[:])
            iotar = cp.tile([P, P], bf16, tag="iotar", name="iotar")
            nc.sync.dma_start(iotar[:], ir_t[:])

            def group_body(g):
                blob = sb.tile([P, WBLOB], bf16, tag="blob", name="blob")
                nc.sync.dma_start(blob[:], stage_t[g])
                oh_all = sb.tile([P, CHUNKS, P], bf16, tag="oh_all", name="oh_all")
                nc.vector.tensor_tensor(
                    out=oh_all[:],
                    in0=iotar[:][:, None, :].to_broadcast([P, CHUNKS, P]),
                    in1=blob[:, OFF_DST:OFF_DST + CHUNKS][:, :, None]
                        .to_broadcast([P, CHUNKS, P]),
                    op=Alu.is_equal,
                )
                psXR = pA.tile([P, HC], f32, tag="A", name="psXR", space="PSUM")
                nc.tensor.matmul(psXR[:], lhsT=blob[:, OFF_XOT:OFF_XOT + P],
                                 rhs=wr[:], start=True, stop=True)
                xr = sb.tile([P, HC], bf16, tag="xr", name="xr")
                nc.vector.tensor_tensor(out=xr[:], in0=psXR[:], in1=brep[:], op=Alu.add)

                xls_all = wd.tile([P, CHUNKS, HC], bf16, tag="xls", name="xls_all")
                z0_all = wd.tile([P, CHUNKS, HC], bf16, tag="z0", name="z0_all")
                for c in range(CHUNKS):
                    psa = pA.tile([P, HC], f32, tag="A", name="psa", space="PSUM")
                    nc.tensor.matmul(psa[:], lhsT=blob[:, OFF_EAT + c * P:OFF_EAT + (c + 1) * P],
                                     rhs=we[:], start=True, stop=False)
                    nc.tensor.matmul(psa[:], lhsT=blob[:, OFF_OHT + c * P:OFF_OHT + (c + 1) * P],
                                     rhs=xr[:], start=False, stop=True)
                    psb = pB.tile([P, HC], f32, tag="B", name="psb", space="PSUM")
                    nc.tensor.matmul(psb[:], lhsT=blob[:, OFF_XST + c * P:OFF_XST + (c + 1) * P],
                                     rhs=wl[:], start=True, stop=True)
                    nc.scalar.copy(xls_all[:, c, :], psb[:])
                    nc.vector.tensor_tensor(out=z0_all[:, c, :], in0=psa[:],
                                            in1=xls_all[:, c, :], op=Alu.add)

                z_all = wd.tile([P, CHUNKS, HC], bf16, tag="z", name="z_all")
                nc.scalar.activation(z_all[:], z0_all[:], Act.Prelu, alpha=NEG_SLOPE)
                zs_all = wd.tile([P, CHUNKS, HC], bf16, tag="zs", name="zs_all")
                nc.gpsimd.tensor_tensor(
                    out=zs_all[:], in0=z_all[:], in1=attw[:], op=Alu.mult,
                )
                score_all = wd.tile([P, CHUNKS, H], f32, tag="score", name="score_all")
                nc.vector.tensor_reduce(
                    out=score_all[:],
                    in_=zs_all[:].rearrange("p t (h c) -> p t h c", h=H),
                    axis=mybir.AxisListType.X, op=Alu.add,
                )
                mp_all = wd.tile([P, CHUNKS, MPW], bf16, tag="mp", name="mp_all")
                nc.scalar.activation(mp_all[:, :, HC:MPW], score_all[:], Act.Exp)
                nc.vector.tensor_tensor(
                    out=mp_all[:, :, :HC].rearrange("p t (h c) -> p t h c", h=H),
                    in0=xls_all[:].rearrange("p t (h c) -> p t h c", h=H),
                    in1=mp_all[:, :, HC:MPW][:, :, :, None]
                        .to_broadcast([P, CHUNKS, H, C]),
                    op=Alu.mult,
                )
                psc = pC.tile([P, MPW], f32, tag="C", name="psc", space="PSUM")
                for c in range(CHUNKS):
                    nc.tensor.matmul(
                        psc[:], lhsT=oh_all[:, c, :], rhs=mp_all[:, c, :],
                        start=(c == 0), stop=(c == CHUNKS - 1),
                    )
                outsb = sb.tile([P, MPW], f32, tag="outsb", name="outsb")
                nc.scalar.copy(outsb[:], psc[:])
                nc.sync.dma_start(out_t[g], outsb[:])

            def stage():
                h = NG // 2
                nc.sync.dma_start(stage_t[0:h], blob_t[0:h])
                nc.sync.dma_start(stage_t[h:NG], blob_t[h:NG])

            def body():
                for _ in range(dup):
                    for g in range(NG):
                        group_body(g)

            stage()
            if reps == 1:
                body()
            else:
                with tc.For_i(0, reps, 1):
                    body()

    nc.compile()
    return nc


def _prep(x, edge_index, edge_attr, batch, bn_gamma, bn_beta, bn_mean, bn_var,
          W_l, b_l, W_r, b_r, W_e, att):
    """Host-side sharding / layout prep.

    Returns (NG, in_maps, cores, perm, cum, fold) where `fold` carries the
    BN-folded weights/biases needed by the host epilogue.
    """
    src = np.asarray(edge_index[0], dtype=np.int64)
    dst = np.asarray(edge_index[1], dtype=np.int64)
    x = np.asarray(x, dtype=np.float32)
    ea = np.asarray(edge_attr, dtype=np.float32)

    rs = 1.0 / np.sqrt(np.asarray(bn_var, np.float64) + EPS)
    s = (rs * np.asarray(bn_gamma, np.float64)).astype(np.float32)
    t = (np.asarray(bn_beta, np.float64) - np.asarray(bn_mean, np.float64) * rs
         * np.asarray(bn_gamma, np.float64)).astype(np.float32)
    Wl = (s[:, None] * np.asarray(W_l, np.float32)).astype(np.float32)
    Wr = (s[:, None] * np.asarray(W_r, np.float32)).astype(np.float32)
    bl = (t @ np.asarray(W_l, np.float32) + np.asarray(b_l, np.float32)).astype(np.float32)
    br = (t @ np.asarray(W_r, np.float32) + np.asarray(b_r, np.float32)).astype(np.float32)

    perm = np.argsort(dst, kind="stable")
    dst_s = dst[perm]
    src_s = src[perm]
    deg = np.bincount(dst, minlength=N)
    cum = np.zeros(N + 1, dtype=np.int64)
    np.cumsum(deg, out=cum[1:])

    cores = []
    NG = 0
    for cid in range(NCORES):
        lo, hi = cid * NODES_PER_CORE, (cid + 1) * NODES_PER_CORE
        groups = []
        n0 = lo
        while n0 < hi:
            span, esum = 0, 0
            while n0 + span < hi and span < SPAN:
                d = int(deg[n0 + span])
                if esum + d > GE and span > 0:
                    break
                assert d <= GE, "node degree exceeds group capacity"
                esum += d
                span += 1
            groups.append((n0, span, esum))
            n0 += span
        cores.append(groups)
        NG = max(NG, len(groups))

    xb = x.astype(BF16)
    xg = xb[src_s]  # [E,128] gathered source features (host gather)
    eab = ea.astype(BF16)[perm]
    jj = np.arange(P, dtype=np.int64)

    in_maps = []
    for cid in range(NCORES):
        groups = cores[cid]
        blob = np.zeros((NG, P, WBLOB), BF16)
        for g, (n0, span, esum) in enumerate(groups):
            e0, e1 = cum[n0], cum[n0 + span]
            dl = np.full(GE, 127, np.int64)
            dl[:esum] = dst_s[e0:e1] - n0
            eat = np.zeros((GE, EDIM), BF16)
            eat[:esum] = eab[e0:e1]
            xst = np.zeros((GE, FIN), BF16)
            xst[:esum] = xg[e0:e1]
            # [2048,128] -> [f, c*128+e]
            blob[g, :, OFF_EAT:OFF_EAT + GE] = (
                eat.reshape(CHUNKS, P, EDIM).transpose(2, 0, 1).reshape(P, GE))
            blob[g, :, OFF_XST:OFF_XST + GE] = (
                xst.reshape(CHUNKS, P, FIN).transpose(2, 0, 1).reshape(P, GE))
            blob[g, :, OFF_OHT:OFF_OHT + GE] = (jj[:, None] == dl[None, :]).astype(BF16)
            blob[g, :, OFF_XOT:OFF_XOT + span] = xb[n0:n0 + span].T
            blob[g, :, OFF_DST:OFF_DST + CHUNKS] = dl.reshape(CHUNKS, P).T.astype(BF16)
        im = dict(
            blob=blob,
            wl=Wl.astype(BF16), wr=Wr.astype(BF16),
            we=np.asarray(W_e, np.float32).astype(BF16),
            attrep=np.tile(np.asarray(att, np.float32).reshape(1, HC), (P, 1)).astype(BF16),
            brep=np.tile((bl + br).reshape(1, HC), (P, 1)).astype(np.float32),
            iotar=np.tile(np.arange(P, dtype=np.float32)[None, :], (P, 1)).astype(BF16),
        )
        in_maps.append(im)
    fold = dict(Wl=Wl, Wr=Wr, bl=bl, br=br)
    return NG, in_maps, cores, perm, cum, fold


def kernel(x, edge_index, edge_attr, batch,
           bn_gamma, bn_beta, bn_mean, bn_var,
           W_l, b_l, W_r, b_r, W_e, att, bias_gat,
           W_head, b_head):
    from concourse.bass_utils import run_bass_kernel_spmd

    x = np.asarray(x, np.float32)
    ea = np.asarray(edge_attr, np.float32)
    batch = np.asarray(batch, np.int64)
    att = np.asarray(att, np.float32)
    We = np.asarray(W_e, np.float32)

    NG, in_maps, cores, perm, cum, fold = _prep(
        x, edge_index, edge_attr, batch, bn_gamma, bn_beta, bn_mean, bn_var,
        W_l, b_l, W_r, b_r, W_e, att)

    for im in in_maps:
        im["attw"] = np.tile(im["attrep"][:, None, :], (1, CHUNKS, 1))

    if (NG, 1) not in _compiled:
        _compiled[(NG, 1)] = _build_nc(NG, 1)
    nc = _compiled[(NG, 1)]
    res = run_bass_kernel_spmd(nc, in_maps, core_ids=list(range(NCORES)))
    gp = [r["gpart"] for r in res.results]  # 8 x [NG, 128, 330]

    num_dev = np.zeros((N, HC), np.float32)
    den_dev = np.zeros((N, H), np.float32)
    for cid in range(NCORES):
        for g, (n0, span, esum) in enumerate(cores[cid]):
            num_dev[n0:n0 + span] = gp[cid][g, :span, :HC]
            den_dev[n0:n0 + span] = gp[cid][g, :span, HC:MPW]

    # ---- host epilogue: self-loop path + normalize + pool + head ----
    # Wl/Wr are BN-folded: they act on RAW x (bias terms bl/br separate)
    Wl, Wr, bl, br = fold["Wl"], fold["Wr"], fold["bl"], fold["br"]
    xl = x @ Wl                      # [N, HC] (no bias)
    xr = x @ Wr
    ea_s = ea[perm]
    S = np.zeros((E + 1, EDIM), np.float64)
    np.cumsum(ea_s, axis=0, out=S[1:])
    sum_attr = (S[cum[1:]] - S[cum[:-1]]).astype(np.float32)
    deg = np.diff(cum).astype(np.float32)
    loop_attr = sum_attr / np.maximum(deg, 1.0)[:, None]
    e_self = loop_attr @ We
    z_self = xl + xr + e_self + (bl + br)
    z_self = np.where(z_self > 0, z_self, NEG_SLOPE * z_self)
    score_self = (z_self.reshape(N, H, C) * att[None]).sum(2)   # [N, H]
    es_self = np.exp(score_self)
    num = num_dev.reshape(N, H, C) + es_self[:, :, None] * xl.reshape(N, H, C)
    den = den_dev + es_self
    outn = (num / den[:, :, None]).mean(1) + bl.reshape(H, C).mean(0) \
        + np.asarray(bias_gat, np.float32)
    outn = np.maximum(outn, 0.0)

    Sg = np.zeros((N + 1, C), np.float64)
    np.cumsum(outn, axis=0, out=Sg[1:])
    bnd = np.searchsorted(batch, np.arange(B + 1))
    gsum = (Sg[bnd[1:]] - Sg[bnd[:-1]]).astype(np.float32)
    gcnt = np.diff(bnd).astype(np.float32)
    g = gsum / np.maximum(gcnt, 1.0)[:, None]
    out = g @ np.asarray(W_head, np.float32) + np.asarray(b_head, np.float32)
    return out.astype(np.float32)
